# revision 8
# baseline (speedup 1.0000x reference)
"""Trainium2 Bass kernel for a 6-layer causal decoder transformer.

Model: B=128, T=256, E=384, H=6, D=64, DFF=1536, L=6, V=65 (f32 reference).
Sharding: pure data-parallel over batch across 8 NeuronCores (16 batches
per core), parameters replicated, no collectives.

v2 over the v1 baseline:
  - FFN (W1/W2) and attention out-proj run in fp8e4m3 with DoubleRow perf
    mode (2 contraction tiles per instruction, 0.5 cyc/row): E-contraction
    operands are zero-padded to 4 chunks, W2's K=1536 is 6 native pairs.
  - Per-(batch,head) softmax consolidated: one [128,3,128] scores PSUM
    tile, two exp+accum activations, one batched reciprocal, normalize ops
    round-robined over DVE/Pool, three P-transposes land in one PSUM tile
    and evacuate with a single copy (round-robined DVE/Act).
  - LN transposes likewise consolidated to one copy per token tile.
  - LN apply alternates DVE tensor_scalar and ScalarE Identity (scale=rs,
    bias=-mu*rs) to balance engines.
  - Residual adds stay on DVE; Pool (gpsimd) takes SBUF-only work
    (softmax normalize share, fp8 pad-chunk memsets).
"""

import sys
from contextlib import ExitStack

sys.path.insert(0, "/opt/trn_rl_repo")

import numpy as np
import ml_dtypes

import concourse.bass as bass
import concourse.bacc as bacc
import concourse.mybir as mybir
import concourse.tile as tile
from concourse.masks import make_identity
from concourse.bass_utils import run_bass_kernel_spmd

F32 = mybir.dt.float32
BF16 = mybir.dt.bfloat16
FP8 = mybir.dt.float8e4
AF = mybir.ActivationFunctionType
OP = mybir.AluOpType
DR = mybir.MatmulPerfMode.DoubleRow

P = 128
E, DFF, H, D, T, L, V = 384, 1536, 6, 64, 256, 6, 65
B = 128
N_CORES = 8
B_LOC = B // N_CORES          # 16 batches per core
NTOK = B_LOC * T              # 4096 tokens per core
NT = NTOK // P                # 32 token tiles
GROUP = 512                   # tokens per group (2 full batches)
NG = NTOK // GROUP            # 8 groups
TPG = GROUP // P              # 4 token tiles per group
BPG = GROUP // T              # 2 batches per group
EC = E // P                   # 3 feature chunks
EC4 = 4                       # padded feature chunks for fp8 DoubleRow
FC = DFF // P                 # 12 dff chunks
NEG = -1.0e9

# fp8 family toggles (build-time)
FP8_FFN = False
FP8_WO = False

_PROG = None  # (nc, zero_bias)


def _ln_stats_group(nc, stat, x_list, eps=1e-5):
    """bn_stats per tile + batched Newton rsqrt. Returns (mv_g, rs_g, nm_g):
    mv_g[:, i, 0:1] = mean of tile i; rs_g[:, i:i+1] = rsqrt(var_i + eps);
    nm_g[:, i:i+1] = -mean_i * rsqrt_i (ScalarE Identity bias)."""
    n = len(x_list)
    mv_g = stat.tile([P, n, 2], F32, tag="mvg")
    for i, xin in enumerate(x_list):
        st6 = stat.tile([P, 6], F32, tag="bn6")
        nc.vector.bn_stats(out=st6[:], in_=xin)
        nc.vector.bn_aggr(out=mv_g[:, i, :], in_=st6[:])
    var = stat.tile([P, n], F32, tag="vare")
    nc.vector.tensor_scalar_add(var[:], mv_g[:, :, 1], eps)
    u = stat.tile([P, n], F32, tag="ue")
    nc.vector.reciprocal(u[:], var[:])
    lin = stat.tile([P, n], F32, tag="line")
    nc.vector.tensor_scalar(lin[:], var[:], 0.73, 0.32, op0=OP.mult, op1=OP.add)
    rs = stat.tile([P, n], F32, tag="rse")
    nc.vector.tensor_tensor(rs[:], u[:], lin[:], OP.mult)       # seed ~ rsqrt
    t1 = stat.tile([P, n], F32, tag="t1e")
    for _ in range(2):                                          # Newton x2
        nc.vector.tensor_tensor(t1[:], rs[:], rs[:], OP.mult)
        nc.vector.tensor_tensor(t1[:], t1[:], var[:], OP.mult)
        nc.vector.tensor_scalar(t1[:], t1[:], -0.5, 1.5, op0=OP.mult, op1=OP.add)
        nc.vector.tensor_tensor(rs[:], rs[:], t1[:], OP.mult)
    nm = stat.tile([P, n], F32, tag="nme")
    nc.vector.tensor_tensor(nm[:], mv_g[:, :, 0], rs[:], OP.mult)
    nc.vector.tensor_scalar(nm[:], nm[:], -1.0, None, op0=OP.mult)
    return mv_g, rs, nm


def build_program(repeat=1, zero_bias=True):
    nc = bacc.Bacc("TRN2", target_bir_lowering=False, debug=False,
                   num_devices=N_CORES)

    for val in (1e-5,):
        t = nc.alloc_sbuf_tensor(f"const-f32-{val}", [P, 1], F32)
        nc.gpsimd.memset(t.ap(), val)
        nc.const_aps.aps[(F32, val)] = t.ap()
    nc.all_engine_barrier()

    # ---- I/O -------------------------------------------------------------
    oht = nc.dram_tensor("oht", [P, NTOK], BF16, kind="ExternalInput").ap()
    embp = nc.dram_tensor("embp", [P, E], BF16, kind="ExternalInput").ap()
    pose = nc.dram_tensor("pose", [T, E], F32, kind="ExternalInput").ap()
    maskd = nc.dram_tensor("maskd", [P, P], BF16, kind="ExternalInput").ap()
    wq = nc.dram_tensor("wq", [L, E, E], BF16, kind="ExternalInput").ap()
    wk = nc.dram_tensor("wk", [L, E, E], BF16, kind="ExternalInput").ap()
    wv = nc.dram_tensor("wv", [L, E, E], BF16, kind="ExternalInput").ap()
    wl = nc.dram_tensor("wl", [E, V], BF16, kind="ExternalInput").ap()
    if FP8_WO:
        wo8 = nc.dram_tensor("wo8", [L, P, EC4, E], FP8, kind="ExternalInput").ap()
    else:
        wo = nc.dram_tensor("wo", [L, E, E], BF16, kind="ExternalInput").ap()
    if FP8_FFN:
        w18 = nc.dram_tensor("w18", [L, P, EC4, DFF], FP8, kind="ExternalInput").ap()
        w28 = nc.dram_tensor("w28", [L, P, FC, E], FP8, kind="ExternalInput").ap()
    else:
        w1 = nc.dram_tensor("w1", [L, E, DFF], BF16, kind="ExternalInput").ap()
        w2 = nc.dram_tensor("w2", [L, DFF, E], BF16, kind="ExternalInput").ap()
    bqf = nc.dram_tensor("bqf", [L, P, EC], F32, kind="ExternalInput").ap()
    bkf = nc.dram_tensor("bkf", [L, P, EC], F32, kind="ExternalInput").ap()
    c1f = nc.dram_tensor("c1f", [L, P, FC], F32, kind="ExternalInput").ap()
    btm = nc.dram_tensor("btm", [L, 3, P, E], F32, kind="ExternalInput").ap()
    blr = nc.dram_tensor("blr", [P, V], F32, kind="ExternalInput").ap()
    out = nc.dram_tensor("out", [NTOK, V], F32, kind="ExternalOutput").ap()

    H2DT = FP8 if FP8_FFN else BF16   # LN2 output dtype (feeds W1)
    HFDT = FP8 if FP8_FFN else BF16   # FFN hidden dtype (feeds W2)
    ODT = FP8 if FP8_WO else BF16     # attention output dtype (feeds Wo)

    with tile.TileContext(nc) as tc, ExitStack() as es:
            ep = es.enter_context
            const = ep(tc.tile_pool(name="const", bufs=1))
            xres = ep(tc.tile_pool(name="xres", bufs=1))
            wa = ep(tc.tile_pool(name="wa", bufs=2))
            wf = ep(tc.tile_pool(name="wf", bufs=2))
            bias = ep(tc.tile_pool(name="bias", bufs=2))
            grp = ep(tc.tile_pool(name="grp", bufs=2))
            grp1 = ep(tc.tile_pool(name="grp1", bufs=1))
            vt = ep(tc.tile_pool(name="vt", bufs=2))
            tk = ep(tc.tile_pool(name="tk", bufs=4))
            bh = ep(tc.tile_pool(name="bh", bufs=6))
            stat = ep(tc.tile_pool(name="stat", bufs=8))
            psmm = ep(tc.tile_pool(name="psmm", bufs=2, space="PSUM"))
            pstr = ep(tc.tile_pool(name="pstr", bufs=2, space="PSUM"))
            pss = ep(tc.tile_pool(name="pss", bufs=2, space="PSUM"))
            psav = ep(tc.tile_pool(name="psav", bufs=2, space="PSUM"))
            # ---- constants ----
            id_bf = const.tile([P, P], BF16, tag="id_bf")
            make_identity(nc, id_bf)
            mask_sb = const.tile([P, P], BF16, tag="mask")
            nc.sync.dma_start(mask_sb[:], maskd[:])
            emb_sb = const.tile([P, E], BF16, tag="emb")
            nc.sync.dma_start(emb_sb[:], embp[:])
            pose_sb = const.tile([P, 2, E], F32, tag="pose")
            nc.sync.dma_start(pose_sb[:, 0, :], pose[0:P, :])
            nc.sync.dma_start(pose_sb[:, 1, :], pose[P : 2 * P, :])
            wl_sb = const.tile([P, EC, V], BF16, tag="wl")
            nc.sync.dma_start(wl_sb[:], wl.rearrange("(kc p) n -> p kc n", p=P))
            blr_sb = const.tile([P, V], F32, tag="blr")
            nc.sync.dma_start(blr_sb[:], blr[:])
            oht_sb = const.tile([P, NTOK], BF16, tag="oht")
            nc.sync.dma_start(oht_sb[:], oht[:])

            # round-robin counters
            _c_ln = [0]     # ln apply DVE/Act
            _c_tc = [0]     # transpose copy-outs DVE/Act
            _c_nm = [0]     # softmax normalize DVE/Pool
            _c_ev = [0]     # psum evacuations DVE/Act

            def rr_copy(dst, src):
                """PSUM->SBUF copy, alternating DVE/Act."""
                if _c_ev[0] % 2 == 0:
                    nc.vector.tensor_copy(dst, src)
                else:
                    nc.scalar.copy(dst, src)
                _c_ev[0] += 1

            def tr_copy(dst, src):
                if _c_tc[0] % 2 == 0:
                    nc.vector.tensor_copy(dst, src)
                else:
                    nc.scalar.copy(dst, src)
                _c_tc[0] += 1

            x_tm = [xres.tile([P, E], F32, tag=f"x{t}", name=f"x{t}") for t in range(NT)]

            def ln_block(tts, out_fm, pad4, out_dt):
                """LN stats+apply+transpose for the tiles in tts -> out_fm
                feature-major [P, EC(+pad), GROUP]."""
                mv_g, rs_g, nm_g = _ln_stats_group(
                    nc, stat, [x_tm[tt][:] for tt in tts])
                for i, tt in enumerate(tts):
                    xh = tk.tile([P, E], BF16, tag="xhat")
                    if _c_ln[0] % 2 == 0:
                        nc.vector.tensor_scalar(
                            xh[:], x_tm[tt][:], mv_g[:, i, 0:1], rs_g[:, i : i + 1],
                            op0=OP.subtract, op1=OP.mult)
                    else:
                        nc.scalar.activation(
                            xh[:], x_tm[tt][:], AF.Identity,
                            bias=nm_g[:, i : i + 1], scale=rs_g[:, i : i + 1])
                    _c_ln[0] += 1
                    pt = pstr.tile([P, EC, P], BF16, tag="tr")
                    for kc in range(EC):
                        nc.tensor.transpose(pt[:, kc, :],
                                            xh[:, kc * P : (kc + 1) * P], id_bf[:])
                    tr_copy(out_fm[:, 0:EC, i * P : (i + 1) * P], pt[:])
                if pad4:
                    nc.gpsimd.memset(out_fm[:, EC, :], 0.0)

            for _rep in range(repeat):
                # ---- x0 = onehot @ emb + pos ----
                for tt in range(NT):
                    xt = x_tm[tt]
                    pe = psmm.tile([P, GROUP], F32, tag="mm")
                    nc.tensor.matmul(pe[:, :E], oht_sb[:, tt * P : (tt + 1) * P],
                                     emb_sb[:], start=True, stop=True)
                    nc.vector.tensor_tensor(xt[:], pe[:, :E], pose_sb[:, tt % 2, :], OP.add)

                # ---- layers ----
                for l in range(L):
                    wq_sb = wa.tile([P, EC, E], BF16, tag="wq")
                    nc.sync.dma_start(wq_sb[:], wq[l].rearrange("(kc p) n -> p kc n", p=P))
                    wk_sb = wa.tile([P, EC, E], BF16, tag="wk")
                    nc.sync.dma_start(wk_sb[:], wk[l].rearrange("(kc p) n -> p kc n", p=P))
                    wv_sb = wa.tile([P, EC, E], BF16, tag="wv")
                    nc.sync.dma_start(wv_sb[:], wv[l].rearrange("(kc p) n -> p kc n", p=P))
                    if FP8_WO:
                        wo_sb = wa.tile([P, EC4, E], FP8, tag="wo8")
                        nc.sync.dma_start(wo_sb[:], wo8[l])
                    else:
                        wo_sb = wa.tile([P, EC, E], BF16, tag="wo")
                        nc.sync.dma_start(wo_sb[:], wo[l].rearrange("(kc p) n -> p kc n", p=P))
                    if FP8_FFN:
                        w1_sb = wf.tile([P, EC4, DFF], FP8, tag="w18")
                        nc.sync.dma_start(w1_sb[:], w18[l])
                        w2_sb = wf.tile([P, FC, E], FP8, tag="w28")
                        nc.sync.dma_start(w2_sb[:], w28[l])
                    else:
                        w1_sb = wf.tile([P, EC, DFF], BF16, tag="w1")
                        nc.sync.dma_start(w1_sb[:], w1[l].rearrange("(kc p) n -> p kc n", p=P))
                        w2_sb = wf.tile([P, FC, E], BF16, tag="w2")
                        nc.sync.dma_start(w2_sb[:], w2[l].rearrange("(kc p) n -> p kc n", p=P))
                    bq_sb = bias.tile([P, EC], F32, tag="bq")
                    nc.sync.dma_start(bq_sb[:], bqf[l])
                    bk_sb = bias.tile([P, EC], F32, tag="bk")
                    nc.sync.dma_start(bk_sb[:], bkf[l])
                    c1_sb = bias.tile([P, FC], F32, tag="c1")
                    nc.sync.dma_start(c1_sb[:], c1f[l])
                    btm_sb = bias.tile([P, 3, E], F32, tag="btm")
                    nc.sync.dma_start(btm_sb[:], btm[l].rearrange("t p n -> p t n"))

                    for g in range(NG):
                        tts = [g * TPG + i for i in range(TPG)]

                        # -- LN1 + transpose to feature-major --
                        h_fm = grp.tile([P, EC, GROUP], BF16, tag="hfm")
                        ln_block(tts, h_fm, pad4=False, out_dt=BF16)

                        # -- Q, K projections (feature-major out, bf16) --
                        q_fm = grp.tile([P, EC, GROUP], BF16, tag="qfm")
                        k_fm = grp.tile([P, EC, GROUP], BF16, tag="kfm")
                        for dst, wsb, bsb in ((q_fm, wq_sb, bq_sb), (k_fm, wk_sb, bk_sb)):
                            for m in range(EC):
                                pq = psmm.tile([P, GROUP], F32, tag="mm")
                                for kc in range(EC):
                                    nc.tensor.matmul(pq[:], wsb[:, kc, m * P : (m + 1) * P],
                                                     h_fm[:, kc, :],
                                                     start=(kc == 0), stop=(kc == EC - 1))
                                if _c_ev[0] % 2 == 0:
                                    nc.vector.tensor_scalar(dst[:, m, :], pq[:],
                                                            bsb[:, m : m + 1], None,
                                                            op0=OP.add)
                                else:
                                    nc.scalar.activation(dst[:, m, :], pq[:], AF.Identity,
                                                         bias=bsb[:, m : m + 1], scale=1.0)
                                _c_ev[0] += 1

                        # -- V projection (token-major out, one group tile) --
                        v_g = vt.tile([P, TPG, E], BF16, tag="vtm")
                        for i, tt in enumerate(tts):
                            pv = psmm.tile([P, GROUP], F32, tag="mm")
                            for kc in range(EC):
                                nc.tensor.matmul(pv[:, :E], h_fm[:, kc, i * P : (i + 1) * P],
                                                 wv_sb[:, kc, :],
                                                 start=(kc == 0), stop=(kc == EC - 1))
                            if zero_bias:
                                rr_copy(v_g[:, i, :], pv[:, :E])
                            else:
                                nc.vector.tensor_tensor(v_g[:, i, :], pv[:, :E],
                                                        btm_sb[:, 0, :], OP.add)

                        # -- attention --
                        o_fm = grp.tile([P, EC4 if FP8_WO else EC, GROUP], ODT, tag="ofm")
                        if FP8_WO:
                            nc.gpsimd.memset(o_fm[:, EC, :], 0.0)
                        for lb in range(BPG):
                            for j in range(EC):  # head pair -> o_fm chunk j
                                pav = psav.tile([P, T], F32, tag="av")
                                for hh in range(2):
                                    h = 2 * j + hh
                                    ro = (h % 2) * 64
                                    mc = h // 2
                                    q_ap = q_fm[ro : ro + 64, mc, lb * T : (lb + 1) * T]
                                    k_ap = k_fm[ro : ro + 64, mc, lb * T : (lb + 1) * T]

                                    ps = pss.tile([P, EC, P], F32, tag="s")
                                    # q-tile 0 vs keys 0:128 (causal)
                                    nc.tensor.matmul(ps[:, 0, :], q_ap[:, 0:P], k_ap[:, 0:P],
                                                     start=True, stop=False)
                                    nc.tensor.matmul(ps[:, 0, :], id_bf[:], mask_sb[:],
                                                     start=False, stop=True)
                                    # q-tile 1 vs keys 0:256, mask on diagonal
                                    nc.tensor.matmul(ps[:, 1:3, :], q_ap[:, P:T], k_ap[:],
                                                     start=True, stop=False)
                                    nc.tensor.matmul(ps[:, 2, :], id_bf[:], mask_sb[:],
                                                     start=False, stop=True)
                                    p_f = bh.tile([P, EC, P], F32, tag="pf")
                                    sums = stat.tile([P, 2], F32, tag="sum")
                                    nc.scalar.activation(p_f[:, 0, :], ps[:, 0, :], AF.Exp,
                                                         bias=0.0, scale=1.0,
                                                         accum_out=sums[:, 0:1])
                                    nc.scalar.activation(p_f[:, 1:3, :], ps[:, 1:3, :], AF.Exp,
                                                         bias=0.0, scale=1.0,
                                                         accum_out=sums[:, 1:2])
                                    r2 = stat.tile([P, 2], F32, tag="r2")
                                    nc.vector.reciprocal(r2[:], sums[:])
                                    p_b = bh.tile([P, EC, P], BF16, tag="pb")
                                    if _c_nm[0] % 2 == 0:
                                        nc.vector.tensor_scalar_mul(p_b[:, 0, :], p_f[:, 0, :],
                                                                    r2[:, 0:1])
                                        nc.gpsimd.tensor_scalar(p_b[:, 1:3, :], p_f[:, 1:3, :],
                                                                r2[:, 1:2], None, op0=OP.mult)
                                    else:
                                        nc.gpsimd.tensor_scalar(p_b[:, 0, :], p_f[:, 0, :],
                                                                r2[:, 0:1], None, op0=OP.mult)
                                        nc.vector.tensor_scalar_mul(p_b[:, 1:3, :], p_f[:, 1:3, :],
                                                                    r2[:, 1:2])
                                    _c_nm[0] += 1
                                    ptp = pstr.tile([P, EC, P], BF16, tag="tr")
                                    for kc in range(EC):
                                        nc.tensor.transpose(ptp[:, kc, :], p_b[:, kc, :],
                                                            id_bf[:])
                                    ptb = bh.tile([P, EC, P], BF16, tag="ptb")
                                    tr_copy(ptb[:], ptp[:])

                                    vsl = slice(h * 64, (h + 1) * 64)
                                    nc.tensor.matmul(pav[ro : ro + 64, 0:P],
                                                     v_g[:, 2 * lb, vsl], ptb[:, 0, :],
                                                     start=True, stop=True)
                                    nc.tensor.matmul(pav[ro : ro + 64, P:T],
                                                     v_g[:, 2 * lb, vsl], ptb[:, 1, :],
                                                     start=True, stop=False)
                                    nc.tensor.matmul(pav[ro : ro + 64, P:T],
                                                     v_g[:, 2 * lb + 1, vsl], ptb[:, 2, :],
                                                     start=False, stop=True)
                                rr_copy(o_fm[:, j, lb * T : (lb + 1) * T], pav[:])

                        # -- attention out-proj + residual --
                        for i, tt in enumerate(tts):
                            pao = psmm.tile([P, GROUP], F32, tag="mm")
                            if FP8_WO:
                                for c in range(2):
                                    nc.tensor.matmul(
                                        pao[:, :E],
                                        o_fm[:, 2 * c : 2 * c + 2, i * P : (i + 1) * P],
                                        wo_sb[:, 2 * c : 2 * c + 2, :],
                                        start=(c == 0), stop=(c == 1), perf_mode=DR)
                            else:
                                for kc in range(EC):
                                    nc.tensor.matmul(pao[:, :E],
                                                     o_fm[:, kc, i * P : (i + 1) * P],
                                                     wo_sb[:, kc, :],
                                                     start=(kc == 0), stop=(kc == EC - 1))
                            if zero_bias:
                                nc.vector.tensor_tensor(x_tm[tt][:], pao[:, :E], x_tm[tt][:], OP.add)
                            else:
                                t1 = tk.tile([P, E], F32, tag="t1")
                                nc.vector.tensor_tensor(t1[:], pao[:, :E], x_tm[tt][:], OP.add)
                                nc.gpsimd.tensor_tensor(x_tm[tt][:], t1[:], btm_sb[:, 1, :], OP.add)

                        # -- LN2 + transpose --
                        h2_fm = grp.tile([P, EC4 if FP8_FFN else EC, GROUP], H2DT, tag="h2fm")
                        ln_block(tts, h2_fm, pad4=FP8_FFN, out_dt=H2DT)

                        # -- FFN: W1 + relu (feature-major hidden) --
                        hf = grp1.tile([P, FC, GROUP], HFDT, tag="hf")
                        for m in range(FC):
                            pf = psmm.tile([P, GROUP], F32, tag="mm")
                            if FP8_FFN:
                                for c in range(2):
                                    nc.tensor.matmul(
                                        pf[:],
                                        w1_sb[:, 2 * c : 2 * c + 2, m * P : (m + 1) * P],
                                        h2_fm[:, 2 * c : 2 * c + 2, :],
                                        start=(c == 0), stop=(c == 1), perf_mode=DR)
                            else:
                                for kc in range(EC):
                                    nc.tensor.matmul(pf[:], w1_sb[:, kc, m * P : (m + 1) * P],
                                                     h2_fm[:, kc, :],
                                                     start=(kc == 0), stop=(kc == EC - 1))
                            if _c_ev[0] % 2 == 0:
                                nc.vector.tensor_scalar(hf[:, m, :], pf[:],
                                                        c1_sb[:, m : m + 1], 0.0,
                                                        op0=OP.add, op1=OP.max)
                            else:
                                nc.scalar.activation(hf[:, m, :], pf[:], AF.Relu,
                                                     bias=c1_sb[:, m : m + 1], scale=1.0)
                            _c_ev[0] += 1

                        # -- W2 + residual --
                        for i, tt in enumerate(tts):
                            pw2 = psmm.tile([P, GROUP], F32, tag="mm")
                            if FP8_FFN:
                                for c in range(FC // 2):
                                    nc.tensor.matmul(
                                        pw2[:, :E],
                                        hf[:, 2 * c : 2 * c + 2, i * P : (i + 1) * P],
                                        w2_sb[:, 2 * c : 2 * c + 2, :],
                                        start=(c == 0), stop=(c == FC // 2 - 1),
                                        perf_mode=DR)
                            else:
                                for kc in range(FC):
                                    nc.tensor.matmul(pw2[:, :E],
                                                     hf[:, kc, i * P : (i + 1) * P],
                                                     w2_sb[:, kc, :],
                                                     start=(kc == 0), stop=(kc == FC - 1))
                            if zero_bias:
                                nc.vector.tensor_tensor(x_tm[tt][:], pw2[:, :E], x_tm[tt][:], OP.add)
                            else:
                                t2 = tk.tile([P, E], F32, tag="t1")
                                nc.vector.tensor_tensor(t2[:], pw2[:, :E], x_tm[tt][:], OP.add)
                                nc.gpsimd.tensor_tensor(x_tm[tt][:], t2[:], btm_sb[:, 2, :], OP.add)

                # ---- final logits ----
                for tt in range(NT):
                    xb = tk.tile([P, E], BF16, tag="xhat")
                    nc.any.tensor_copy(out=xb[:], in_=x_tm[tt][:])
                    ptx = pstr.tile([P, EC, P], BF16, tag="tr")
                    for kc in range(EC):
                        nc.tensor.transpose(ptx[:, kc, :], xb[:, kc * P : (kc + 1) * P],
                                            id_bf[:])
                    xf = tk.tile([P, EC, P], BF16, tag="xf")
                    tr_copy(xf[:], ptx[:])
                    pl = psmm.tile([P, GROUP], F32, tag="mm")
                    for kc in range(EC):
                        nc.tensor.matmul(pl[:, :V], xf[:, kc, :], wl_sb[:, kc, :],
                                         start=(kc == 0), stop=(kc == EC - 1))
                    lg = tk.tile([P, V], F32, tag="lg")
                    if zero_bias:
                        rr_copy(lg[:], pl[:, :V])
                    else:
                        nc.vector.tensor_tensor(lg[:], pl[:, :V], blr_sb[:], OP.add)
                    nc.sync.dma_start(out[tt * P : (tt + 1) * P, :], lg[:])

    nc.compile()
    return nc


def _prep_host(inputs):
    f32 = np.float32
    bf16 = ml_dtypes.bfloat16
    fp8 = ml_dtypes.float8_e4m3
    tokens = np.asarray(inputs["tokens"]).astype(np.int64)
    emb = np.asarray(inputs["emb"], dtype=f32)
    pos_enc = np.asarray(inputs["pos_enc"], dtype=f32)
    Wq = np.asarray(inputs["Wq"], dtype=f32)
    Wk = np.asarray(inputs["Wk"], dtype=f32)
    Wv = np.asarray(inputs["Wv"], dtype=f32)
    Wo = np.asarray(inputs["Wo"], dtype=f32)
    W1 = np.asarray(inputs["W1"], dtype=f32)
    W2 = np.asarray(inputs["W2"], dtype=f32)
    Wl = np.asarray(inputs["Wl"], dtype=f32)
    bq = np.asarray(inputs["bq"], dtype=f32)
    bk = np.asarray(inputs["bk"], dtype=f32)
    bv = np.asarray(inputs["bv"], dtype=f32)
    bo = np.asarray(inputs["bo"], dtype=f32)
    c1 = np.asarray(inputs["c1"], dtype=f32)
    c2 = np.asarray(inputs["c2"], dtype=f32)
    bl = np.asarray(inputs["bl"], dtype=f32)
    g1 = np.asarray(inputs["ln1_g"], dtype=f32)
    b1 = np.asarray(inputs["ln1_b"], dtype=f32)
    g2 = np.asarray(inputs["ln2_g"], dtype=f32)
    b2 = np.asarray(inputs["ln2_b"], dtype=f32)

    scale = D ** -0.5
    wq_f = np.empty((L, E, E), f32)
    wk_f = np.empty((L, E, E), f32)
    wv_f = np.empty((L, E, E), f32)
    w1_f = np.empty((L, E, DFF), f32)
    bq_f = np.empty((L, E), f32)
    bk_f = np.empty((L, E), f32)
    bv_f = np.empty((L, E), f32)
    c1_f = np.empty((L, DFF), f32)
    for l in range(L):
        wq_f[l] = g1[l][:, None] * Wq[l] * scale
        bq_f[l] = (b1[l] @ Wq[l] + bq[l]) * scale
        wk_f[l] = g1[l][:, None] * Wk[l]
        bk_f[l] = b1[l] @ Wk[l] + bk[l]
        wv_f[l] = g1[l][:, None] * Wv[l]
        bv_f[l] = b1[l] @ Wv[l] + bv[l]
        w1_f[l] = g2[l][:, None] * W1[l]
        c1_f[l] = b2[l] @ W1[l] + c1[l]

    common = {
        "embp": np.zeros((P, E), bf16),
        "pose": pos_enc,
        "maskd": np.where(np.tril(np.ones((P, P), bool)), 0.0, NEG).astype(bf16),
        "wq": wq_f.astype(bf16),
        "wk": wk_f.astype(bf16),
        "wv": wv_f.astype(bf16),
        "wl": Wl.astype(bf16),
        "bqf": np.ascontiguousarray(bq_f.reshape(L, EC, P).transpose(0, 2, 1)),
        "bkf": np.ascontiguousarray(bk_f.reshape(L, EC, P).transpose(0, 2, 1)),
        "c1f": np.ascontiguousarray(c1_f.reshape(L, FC, P).transpose(0, 2, 1)),
        "btm": np.ascontiguousarray(
            np.broadcast_to(
                np.stack([bv_f, bo, c2], axis=1)[:, :, None, :], (L, 3, P, E)
            )
        ).astype(f32),
        "blr": np.broadcast_to(bl[None, :], (P, V)).astype(f32),
    }
    common["embp"][:V, :] = emb.astype(bf16)

    if FP8_WO:
        wo8 = np.zeros((L, P, EC4, E), f32)
        wo8[:, :, 0:EC, :] = Wo.reshape(L, EC, P, E).transpose(0, 2, 1, 3)
        common["wo8"] = wo8.astype(fp8)
    else:
        common["wo"] = Wo.astype(bf16)
    if FP8_FFN:
        w18 = np.zeros((L, P, EC4, DFF), f32)
        w18[:, :, 0:EC, :] = w1_f.reshape(L, EC, P, DFF).transpose(0, 2, 1, 3)
        common["w18"] = w18.astype(fp8)
        common["w28"] = np.ascontiguousarray(
            W2.reshape(L, FC, P, E).transpose(0, 2, 1, 3)).astype(fp8)
    else:
        common["w1"] = w1_f.astype(bf16)
        common["w2"] = W2.astype(bf16)

    in_maps = []
    for c in range(N_CORES):
        tok_c = tokens[c * B_LOC : (c + 1) * B_LOC].reshape(-1)
        oht = np.zeros((P, NTOK), bf16)
        oht[tok_c, np.arange(NTOK)] = 1
        m = dict(common)
        m["oht"] = oht
        in_maps.append(m)
    return in_maps


def _biases_all_zero(inputs):
    zs = [inputs[k] for k in ("bq", "bk", "bv", "bo", "c1", "c2", "bl",
                              "ln1_b", "ln2_b")]
    return all(not np.any(np.asarray(z)) for z in zs)


def kernel(**inputs) -> np.ndarray:
    global _PROG
    zb = _biases_all_zero(inputs)
    if _PROG is None or _PROG[1] != zb:
        _PROG = (build_program(zero_bias=zb), zb)
    nc = _PROG[0]
    in_maps = _prep_host(inputs)
    res = run_bass_kernel_spmd(nc, in_maps, list(range(N_CORES)))
    outs = [res.results[c]["out"].reshape(B_LOC, T, V) for c in range(N_CORES)]
    return np.concatenate(outs, axis=0).astype(np.float32)


# revision 9
# speedup vs baseline: 1.4691x; 1.4691x over previous
"""Trainium2 Bass kernel for a 6-layer causal decoder transformer.

Model: B=128, T=256, E=384, H=6, D=64, DFF=1536, L=6, V=65 (f32 reference).
Sharding: pure data-parallel over batch across 8 NeuronCores (16 batches
per core), parameters replicated, no collectives.

v2 over the v1 baseline:
  - FFN (W1/W2) and attention out-proj run in fp8e4m3 with DoubleRow perf
    mode (2 contraction tiles per instruction, 0.5 cyc/row): E-contraction
    operands are zero-padded to 4 chunks, W2's K=1536 is 6 native pairs.
  - Per-(batch,head) softmax consolidated: one [128,3,128] scores PSUM
    tile, two exp+accum activations, one batched reciprocal, normalize ops
    round-robined over DVE/Pool, three P-transposes land in one PSUM tile
    and evacuate with a single copy (round-robined DVE/Act).
  - LN transposes likewise consolidated to one copy per token tile.
  - LN apply alternates DVE tensor_scalar and ScalarE Identity (scale=rs,
    bias=-mu*rs) to balance engines.
  - Residual adds stay on DVE; Pool (gpsimd) takes SBUF-only work
    (softmax normalize share, fp8 pad-chunk memsets).
"""

import sys
from contextlib import ExitStack

sys.path.insert(0, "/opt/trn_rl_repo")

import numpy as np
import ml_dtypes

import concourse.bass as bass
import concourse.bacc as bacc
import concourse.mybir as mybir
import concourse.tile as tile
from concourse.masks import make_identity
from concourse.bass_utils import run_bass_kernel_spmd

F32 = mybir.dt.float32
BF16 = mybir.dt.bfloat16
FP8 = mybir.dt.float8e4
AF = mybir.ActivationFunctionType
OP = mybir.AluOpType
DR = mybir.MatmulPerfMode.DoubleRow

P = 128
E, DFF, H, D, T, L, V = 384, 1536, 6, 64, 256, 6, 65
B = 128
N_CORES = 8
B_LOC = B // N_CORES          # 16 batches per core
NTOK = B_LOC * T              # 4096 tokens per core
NT = NTOK // P                # 32 token tiles
GROUP = 512                   # tokens per group (2 full batches)
NG = NTOK // GROUP            # 8 groups
TPG = GROUP // P              # 4 token tiles per group
BPG = GROUP // T              # 2 batches per group
EC = E // P                   # 3 feature chunks
EC4 = 4                       # padded feature chunks for fp8 DoubleRow
FC = DFF // P                 # 12 dff chunks
NEG = -1.0e9

# fp8 family toggles (build-time)
FP8_FFN = False
FP8_WO = False

_PROG = None  # (nc, zero_bias)


def _ln_stats_group(nc, stat, x_list, eps=1e-5):
    """bn_stats per tile + batched Newton rsqrt. Returns (mv_g, rs_g, nm_g):
    mv_g[:, i, 0:1] = mean of tile i; rs_g[:, i:i+1] = rsqrt(var_i + eps);
    nm_g[:, i:i+1] = -mean_i * rsqrt_i (ScalarE Identity bias)."""
    n = len(x_list)
    mv_g = stat.tile([P, n, 2], F32, tag="mvg")
    for i, xin in enumerate(x_list):
        st6 = stat.tile([P, 6], F32, tag="bn6")
        nc.vector.bn_stats(out=st6[:], in_=xin)
        nc.vector.bn_aggr(out=mv_g[:, i, :], in_=st6[:])
    var = stat.tile([P, n], F32, tag="vare")
    nc.vector.tensor_scalar_add(var[:], mv_g[:, :, 1], eps)
    u = stat.tile([P, n], F32, tag="ue")
    nc.vector.reciprocal(u[:], var[:])
    lin = stat.tile([P, n], F32, tag="line")
    nc.vector.tensor_scalar(lin[:], var[:], 0.73, 0.32, op0=OP.mult, op1=OP.add)
    rs = stat.tile([P, n], F32, tag="rse")
    nc.vector.tensor_tensor(rs[:], u[:], lin[:], OP.mult)       # seed ~ rsqrt
    t1 = stat.tile([P, n], F32, tag="t1e")
    for _ in range(2):                                          # Newton x2
        nc.vector.tensor_tensor(t1[:], rs[:], rs[:], OP.mult)
        nc.vector.tensor_tensor(t1[:], t1[:], var[:], OP.mult)
        nc.vector.tensor_scalar(t1[:], t1[:], -0.5, 1.5, op0=OP.mult, op1=OP.add)
        nc.vector.tensor_tensor(rs[:], rs[:], t1[:], OP.mult)
    nm = stat.tile([P, n], F32, tag="nme")
    nc.vector.tensor_tensor(nm[:], mv_g[:, :, 0], rs[:], OP.mult)
    nc.vector.tensor_scalar(nm[:], nm[:], -1.0, None, op0=OP.mult)
    return mv_g, rs, nm


def build_program(repeat=1, zero_bias=True):
    nc = bacc.Bacc("TRN2", target_bir_lowering=False, debug=False,
                   num_devices=N_CORES)

    for val in (1e-5,):
        t = nc.alloc_sbuf_tensor(f"const-f32-{val}", [P, 1], F32)
        nc.gpsimd.memset(t.ap(), val)
        nc.const_aps.aps[(F32, val)] = t.ap()
    nc.all_engine_barrier()

    # ---- I/O -------------------------------------------------------------
    oht = nc.dram_tensor("oht", [P, NTOK], BF16, kind="ExternalInput").ap()
    embp = nc.dram_tensor("embp", [P, E], BF16, kind="ExternalInput").ap()
    pose = nc.dram_tensor("pose", [T, E], F32, kind="ExternalInput").ap()
    maskd = nc.dram_tensor("maskd", [P, P], BF16, kind="ExternalInput").ap()
    wq = nc.dram_tensor("wq", [L, E, E], BF16, kind="ExternalInput").ap()
    wk = nc.dram_tensor("wk", [L, E, E], BF16, kind="ExternalInput").ap()
    wv = nc.dram_tensor("wv", [L, E, E], BF16, kind="ExternalInput").ap()
    wl = nc.dram_tensor("wl", [E, V], BF16, kind="ExternalInput").ap()
    if FP8_WO:
        wo8 = nc.dram_tensor("wo8", [L, P, EC4, E], FP8, kind="ExternalInput").ap()
    else:
        wo = nc.dram_tensor("wo", [L, E, E], BF16, kind="ExternalInput").ap()
    if FP8_FFN:
        w18 = nc.dram_tensor("w18", [L, P, EC4, DFF], FP8, kind="ExternalInput").ap()
        w28 = nc.dram_tensor("w28", [L, P, FC, E], FP8, kind="ExternalInput").ap()
    else:
        w1 = nc.dram_tensor("w1", [L, E, DFF], BF16, kind="ExternalInput").ap()
        w2 = nc.dram_tensor("w2", [L, DFF, E], BF16, kind="ExternalInput").ap()
    bqf = nc.dram_tensor("bqf", [L, P, EC], F32, kind="ExternalInput").ap()
    bkf = nc.dram_tensor("bkf", [L, P, EC], F32, kind="ExternalInput").ap()
    c1f = nc.dram_tensor("c1f", [L, P, FC], F32, kind="ExternalInput").ap()
    btm = nc.dram_tensor("btm", [L, 3, P, E], F32, kind="ExternalInput").ap()
    blr = nc.dram_tensor("blr", [P, V], F32, kind="ExternalInput").ap()
    out = nc.dram_tensor("out", [NTOK, V], F32, kind="ExternalOutput").ap()

    H2DT = FP8 if FP8_FFN else BF16   # LN2 output dtype (feeds W1)
    HFDT = FP8 if FP8_FFN else BF16   # FFN hidden dtype (feeds W2)
    ODT = FP8 if FP8_WO else BF16     # attention output dtype (feeds Wo)

    with tile.TileContext(nc) as tc, ExitStack() as es:
            ep = es.enter_context
            const = ep(tc.tile_pool(name="const", bufs=1))
            xres = ep(tc.tile_pool(name="xres", bufs=1))
            wa = ep(tc.tile_pool(name="wa", bufs=2))
            wf = ep(tc.tile_pool(name="wf", bufs=2))
            bias = ep(tc.tile_pool(name="bias", bufs=2))
            grp = ep(tc.tile_pool(name="grp", bufs=2))
            grp1 = ep(tc.tile_pool(name="grp1", bufs=1))
            vt = ep(tc.tile_pool(name="vt", bufs=2))
            tk = ep(tc.tile_pool(name="tk", bufs=4))
            bh = ep(tc.tile_pool(name="bh", bufs=6))
            stat = ep(tc.tile_pool(name="stat", bufs=8))
            psmm = ep(tc.tile_pool(name="psmm", bufs=2, space="PSUM"))
            pstr = ep(tc.tile_pool(name="pstr", bufs=2, space="PSUM"))
            pss = ep(tc.tile_pool(name="pss", bufs=2, space="PSUM"))
            psav = ep(tc.tile_pool(name="psav", bufs=2, space="PSUM"))
            # ---- constants ----
            id_bf = const.tile([P, P], BF16, tag="id_bf")
            make_identity(nc, id_bf)
            mask_sb = const.tile([P, P], BF16, tag="mask")
            nc.sync.dma_start(mask_sb[:], maskd[:])
            emb_sb = const.tile([P, E], BF16, tag="emb")
            nc.sync.dma_start(emb_sb[:], embp[:])
            pose_sb = const.tile([P, 2, E], F32, tag="pose")
            nc.sync.dma_start(pose_sb[:, 0, :], pose[0:P, :])
            nc.sync.dma_start(pose_sb[:, 1, :], pose[P : 2 * P, :])
            wl_sb = const.tile([P, EC, V], BF16, tag="wl")
            nc.sync.dma_start(wl_sb[:], wl.rearrange("(kc p) n -> p kc n", p=P))
            blr_sb = const.tile([P, V], F32, tag="blr")
            nc.sync.dma_start(blr_sb[:], blr[:])
            oht_sb = const.tile([P, NTOK], BF16, tag="oht")
            nc.sync.dma_start(oht_sb[:], oht[:])

            # round-robin counters
            _c_ln = [0]     # ln apply DVE/Act
            _c_tc = [0]     # transpose copy-outs DVE/Act
            _c_nm = [0]     # softmax normalize DVE/Pool
            _c_ev = [0]     # psum evacuations DVE/Act

            def rr_copy(dst, src):
                """PSUM->SBUF copy, alternating DVE/Act."""
                if _c_ev[0] % 2 == 0:
                    nc.vector.tensor_copy(dst, src)
                else:
                    nc.scalar.copy(dst, src)
                _c_ev[0] += 1

            def tr_copy(dst, src):
                if _c_tc[0] % 2 == 0:
                    nc.vector.tensor_copy(dst, src)
                else:
                    nc.scalar.copy(dst, src)
                _c_tc[0] += 1

            x_tm = [xres.tile([P, E], F32, tag=f"x{t}", name=f"x{t}") for t in range(NT)]

            def ln_block(tts, out_fm, pad4, out_dt):
                """LN stats+apply+transpose for the tiles in tts -> out_fm
                feature-major [P, EC(+pad), GROUP]."""
                mv_g, rs_g, nm_g = _ln_stats_group(
                    nc, stat, [x_tm[tt][:] for tt in tts])
                for i, tt in enumerate(tts):
                    xh = tk.tile([P, E], BF16, tag="xhat")
                    if _c_ln[0] % 2 == 0:
                        nc.vector.tensor_scalar(
                            xh[:], x_tm[tt][:], mv_g[:, i, 0:1], rs_g[:, i : i + 1],
                            op0=OP.subtract, op1=OP.mult)
                    else:
                        nc.scalar.activation(
                            xh[:], x_tm[tt][:], AF.Identity,
                            bias=nm_g[:, i : i + 1], scale=rs_g[:, i : i + 1])
                    _c_ln[0] += 1
                    pt = pstr.tile([P, EC, P], BF16, tag="tr")
                    for kc in range(EC):
                        nc.tensor.transpose(pt[:, kc, :],
                                            xh[:, kc * P : (kc + 1) * P], id_bf[:])
                    tr_copy(out_fm[:, 0:EC, i * P : (i + 1) * P], pt[:])
                if pad4:
                    nc.gpsimd.memset(out_fm[:, EC, :], 0.0)

            for _rep in range(repeat):
                # ---- x0 = onehot @ emb + pos ----
                for tt in range(NT):
                    xt = x_tm[tt]
                    pe = psmm.tile([P, GROUP], F32, tag="mm")
                    nc.tensor.matmul(pe[:, :E], oht_sb[:, tt * P : (tt + 1) * P],
                                     emb_sb[:], start=True, stop=True)
                    nc.vector.tensor_tensor(xt[:], pe[:, :E], pose_sb[:, tt % 2, :], OP.add)

                # ---- layers ----
                for l in range(L):
                    wq_sb = wa.tile([P, EC, E], BF16, tag="wq")
                    nc.sync.dma_start(wq_sb[:], wq[l].rearrange("(kc p) n -> p kc n", p=P))
                    wk_sb = wa.tile([P, EC, E], BF16, tag="wk")
                    nc.sync.dma_start(wk_sb[:], wk[l].rearrange("(kc p) n -> p kc n", p=P))
                    wv_sb = wa.tile([P, EC, E], BF16, tag="wv")
                    nc.sync.dma_start(wv_sb[:], wv[l].rearrange("(kc p) n -> p kc n", p=P))
                    if FP8_WO:
                        wo_sb = wa.tile([P, EC4, E], FP8, tag="wo8")
                        nc.sync.dma_start(wo_sb[:], wo8[l])
                    else:
                        wo_sb = wa.tile([P, EC, E], BF16, tag="wo")
                        nc.sync.dma_start(wo_sb[:], wo[l].rearrange("(kc p) n -> p kc n", p=P))
                    if FP8_FFN:
                        w1_sb = wf.tile([P, EC4, DFF], FP8, tag="w18")
                        nc.sync.dma_start(w1_sb[:], w18[l])
                        w2_sb = wf.tile([P, FC, E], FP8, tag="w28")
                        nc.sync.dma_start(w2_sb[:], w28[l])
                    else:
                        w1_sb = wf.tile([P, EC, DFF], BF16, tag="w1")
                        nc.sync.dma_start(w1_sb[:], w1[l].rearrange("(kc p) n -> p kc n", p=P))
                        w2_sb = wf.tile([P, FC, E], BF16, tag="w2")
                        nc.sync.dma_start(w2_sb[:], w2[l].rearrange("(kc p) n -> p kc n", p=P))
                    bq_sb = bias.tile([P, EC], F32, tag="bq")
                    nc.sync.dma_start(bq_sb[:], bqf[l])
                    bk_sb = bias.tile([P, EC], F32, tag="bk")
                    nc.sync.dma_start(bk_sb[:], bkf[l])
                    c1_sb = bias.tile([P, FC], F32, tag="c1")
                    nc.sync.dma_start(c1_sb[:], c1f[l])
                    btm_sb = bias.tile([P, 3, E], F32, tag="btm")
                    nc.sync.dma_start(btm_sb[:], btm[l].rearrange("t p n -> p t n"))

                    for g in range(NG):
                        tts = [g * TPG + i for i in range(TPG)]

                        # -- LN1 + transpose to feature-major --
                        h_fm = grp.tile([P, EC, GROUP], BF16, tag="hfm")
                        ln_block(tts, h_fm, pad4=False, out_dt=BF16)

                        # -- Q, K projections (feature-major out, bf16) --
                        q_fm = grp.tile([P, EC, GROUP], BF16, tag="qfm")
                        k_fm = grp.tile([P, EC, GROUP], BF16, tag="kfm")
                        for dst, wsb, bsb in ((q_fm, wq_sb, bq_sb), (k_fm, wk_sb, bk_sb)):
                            for m in range(EC):
                                pq = psmm.tile([P, GROUP], F32, tag="mm")
                                for kc in range(EC):
                                    nc.tensor.matmul(pq[:], wsb[:, kc, m * P : (m + 1) * P],
                                                     h_fm[:, kc, :],
                                                     start=(kc == 0), stop=(kc == EC - 1))
                                if _c_ev[0] % 2 == 0:
                                    nc.vector.tensor_scalar(dst[:, m, :], pq[:],
                                                            bsb[:, m : m + 1], None,
                                                            op0=OP.add)
                                else:
                                    nc.scalar.activation(dst[:, m, :], pq[:], AF.Identity,
                                                         bias=bsb[:, m : m + 1], scale=1.0)
                                _c_ev[0] += 1

                        # -- V projection (token-major out, one group tile) --
                        v_g = vt.tile([P, TPG, E], BF16, tag="vtm")
                        for i, tt in enumerate(tts):
                            pv = psmm.tile([P, GROUP], F32, tag="mm")
                            for kc in range(EC):
                                nc.tensor.matmul(pv[:, :E], h_fm[:, kc, i * P : (i + 1) * P],
                                                 wv_sb[:, kc, :],
                                                 start=(kc == 0), stop=(kc == EC - 1))
                            if zero_bias:
                                rr_copy(v_g[:, i, :], pv[:, :E])
                            else:
                                nc.vector.tensor_tensor(v_g[:, i, :], pv[:, :E],
                                                        btm_sb[:, 0, :], OP.add)

                        # -- attention --
                        o_fm = grp.tile([P, EC4 if FP8_WO else EC, GROUP], ODT, tag="ofm")
                        if FP8_WO:
                            nc.gpsimd.memset(o_fm[:, EC, :], 0.0)
                        for lb in range(BPG):
                            for j in range(EC):  # head pair -> o_fm chunk j
                                pav = psav.tile([P, T], F32, tag="av")
                                for hh in range(2):
                                    h = 2 * j + hh
                                    ro = (h % 2) * 64
                                    mc = h // 2
                                    q_ap = q_fm[ro : ro + 64, mc, lb * T : (lb + 1) * T]
                                    k_ap = k_fm[ro : ro + 64, mc, lb * T : (lb + 1) * T]

                                    ps = pss.tile([P, EC, P], F32, tag="s")
                                    # q-tile 0 vs keys 0:128 (causal)
                                    nc.tensor.matmul(ps[:, 0, :], q_ap[:, 0:P], k_ap[:, 0:P],
                                                     start=True, stop=False)
                                    nc.tensor.matmul(ps[:, 0, :], id_bf[:], mask_sb[:],
                                                     start=False, stop=True)
                                    # q-tile 1 vs keys 0:256, mask on diagonal
                                    nc.tensor.matmul(ps[:, 1:3, :], q_ap[:, P:T], k_ap[:],
                                                     start=True, stop=False)
                                    nc.tensor.matmul(ps[:, 2, :], id_bf[:], mask_sb[:],
                                                     start=False, stop=True)
                                    p_f = bh.tile([P, EC, P], F32, tag="pf")
                                    sums = stat.tile([P, 2], F32, tag="sum")
                                    nc.scalar.activation(p_f[:, 0, :], ps[:, 0, :], AF.Exp,
                                                         bias=0.0, scale=1.0,
                                                         accum_out=sums[:, 0:1])
                                    nc.scalar.activation(p_f[:, 1:3, :], ps[:, 1:3, :], AF.Exp,
                                                         bias=0.0, scale=1.0,
                                                         accum_out=sums[:, 1:2])
                                    r2 = stat.tile([P, 2], F32, tag="r2")
                                    nc.vector.reciprocal(r2[:], sums[:])
                                    p_b = bh.tile([P, EC, P], BF16, tag="pb")
                                    nc.vector.tensor_scalar_mul(p_b[:, 0, :], p_f[:, 0, :],
                                                                r2[:, 0:1])
                                    nc.vector.tensor_scalar_mul(p_b[:, 1:3, :], p_f[:, 1:3, :],
                                                                r2[:, 1:2])
                                    _c_nm[0] += 1
                                    ptp = pstr.tile([P, EC, P], BF16, tag="tr")
                                    for kc in range(EC):
                                        nc.tensor.transpose(ptp[:, kc, :], p_b[:, kc, :],
                                                            id_bf[:])
                                    ptb = bh.tile([P, EC, P], BF16, tag="ptb")
                                    tr_copy(ptb[:], ptp[:])

                                    vsl = slice(h * 64, (h + 1) * 64)
                                    nc.tensor.matmul(pav[ro : ro + 64, 0:P],
                                                     v_g[:, 2 * lb, vsl], ptb[:, 0, :],
                                                     start=True, stop=True)
                                    nc.tensor.matmul(pav[ro : ro + 64, P:T],
                                                     v_g[:, 2 * lb, vsl], ptb[:, 1, :],
                                                     start=True, stop=False)
                                    nc.tensor.matmul(pav[ro : ro + 64, P:T],
                                                     v_g[:, 2 * lb + 1, vsl], ptb[:, 2, :],
                                                     start=False, stop=True)
                                rr_copy(o_fm[:, j, lb * T : (lb + 1) * T], pav[:])

                        # -- attention out-proj + residual --
                        for i, tt in enumerate(tts):
                            pao = psmm.tile([P, GROUP], F32, tag="mm")
                            if FP8_WO:
                                for c in range(2):
                                    nc.tensor.matmul(
                                        pao[:, :E],
                                        o_fm[:, 2 * c : 2 * c + 2, i * P : (i + 1) * P],
                                        wo_sb[:, 2 * c : 2 * c + 2, :],
                                        start=(c == 0), stop=(c == 1), perf_mode=DR)
                            else:
                                for kc in range(EC):
                                    nc.tensor.matmul(pao[:, :E],
                                                     o_fm[:, kc, i * P : (i + 1) * P],
                                                     wo_sb[:, kc, :],
                                                     start=(kc == 0), stop=(kc == EC - 1))
                            if zero_bias:
                                nc.vector.tensor_tensor(x_tm[tt][:], pao[:, :E], x_tm[tt][:], OP.add)
                            else:
                                t1 = tk.tile([P, E], F32, tag="t1")
                                nc.vector.tensor_tensor(t1[:], pao[:, :E], x_tm[tt][:], OP.add)
                                nc.gpsimd.tensor_tensor(x_tm[tt][:], t1[:], btm_sb[:, 1, :], OP.add)

                        # -- LN2 + transpose --
                        h2_fm = grp.tile([P, EC4 if FP8_FFN else EC, GROUP], H2DT, tag="h2fm")
                        ln_block(tts, h2_fm, pad4=FP8_FFN, out_dt=H2DT)

                        # -- FFN: W1 + relu (feature-major hidden) --
                        hf = grp1.tile([P, FC, GROUP], HFDT, tag="hf")
                        for m in range(FC):
                            pf = psmm.tile([P, GROUP], F32, tag="mm")
                            if FP8_FFN:
                                for c in range(2):
                                    nc.tensor.matmul(
                                        pf[:],
                                        w1_sb[:, 2 * c : 2 * c + 2, m * P : (m + 1) * P],
                                        h2_fm[:, 2 * c : 2 * c + 2, :],
                                        start=(c == 0), stop=(c == 1), perf_mode=DR)
                            else:
                                for kc in range(EC):
                                    nc.tensor.matmul(pf[:], w1_sb[:, kc, m * P : (m + 1) * P],
                                                     h2_fm[:, kc, :],
                                                     start=(kc == 0), stop=(kc == EC - 1))
                            if _c_ev[0] % 2 == 0:
                                nc.vector.tensor_scalar(hf[:, m, :], pf[:],
                                                        c1_sb[:, m : m + 1], 0.0,
                                                        op0=OP.add, op1=OP.max)
                            else:
                                nc.scalar.activation(hf[:, m, :], pf[:], AF.Relu,
                                                     bias=c1_sb[:, m : m + 1], scale=1.0)
                            _c_ev[0] += 1

                        # -- W2 + residual --
                        for i, tt in enumerate(tts):
                            pw2 = psmm.tile([P, GROUP], F32, tag="mm")
                            if FP8_FFN:
                                for c in range(FC // 2):
                                    nc.tensor.matmul(
                                        pw2[:, :E],
                                        hf[:, 2 * c : 2 * c + 2, i * P : (i + 1) * P],
                                        w2_sb[:, 2 * c : 2 * c + 2, :],
                                        start=(c == 0), stop=(c == FC // 2 - 1),
                                        perf_mode=DR)
                            else:
                                for kc in range(FC):
                                    nc.tensor.matmul(pw2[:, :E],
                                                     hf[:, kc, i * P : (i + 1) * P],
                                                     w2_sb[:, kc, :],
                                                     start=(kc == 0), stop=(kc == FC - 1))
                            if zero_bias:
                                nc.vector.tensor_tensor(x_tm[tt][:], pw2[:, :E], x_tm[tt][:], OP.add)
                            else:
                                t2 = tk.tile([P, E], F32, tag="t1")
                                nc.vector.tensor_tensor(t2[:], pw2[:, :E], x_tm[tt][:], OP.add)
                                nc.gpsimd.tensor_tensor(x_tm[tt][:], t2[:], btm_sb[:, 2, :], OP.add)

                # ---- final logits ----
                for tt in range(NT):
                    xb = tk.tile([P, E], BF16, tag="xhat")
                    nc.any.tensor_copy(out=xb[:], in_=x_tm[tt][:])
                    ptx = pstr.tile([P, EC, P], BF16, tag="tr")
                    for kc in range(EC):
                        nc.tensor.transpose(ptx[:, kc, :], xb[:, kc * P : (kc + 1) * P],
                                            id_bf[:])
                    xf = tk.tile([P, EC, P], BF16, tag="xf")
                    tr_copy(xf[:], ptx[:])
                    pl = psmm.tile([P, GROUP], F32, tag="mm")
                    for kc in range(EC):
                        nc.tensor.matmul(pl[:, :V], xf[:, kc, :], wl_sb[:, kc, :],
                                         start=(kc == 0), stop=(kc == EC - 1))
                    lg = tk.tile([P, V], F32, tag="lg")
                    if zero_bias:
                        rr_copy(lg[:], pl[:, :V])
                    else:
                        nc.vector.tensor_tensor(lg[:], pl[:, :V], blr_sb[:], OP.add)
                    nc.sync.dma_start(out[tt * P : (tt + 1) * P, :], lg[:])

    nc.compile()
    return nc


def _prep_host(inputs):
    f32 = np.float32
    bf16 = ml_dtypes.bfloat16
    fp8 = ml_dtypes.float8_e4m3
    tokens = np.asarray(inputs["tokens"]).astype(np.int64)
    emb = np.asarray(inputs["emb"], dtype=f32)
    pos_enc = np.asarray(inputs["pos_enc"], dtype=f32)
    Wq = np.asarray(inputs["Wq"], dtype=f32)
    Wk = np.asarray(inputs["Wk"], dtype=f32)
    Wv = np.asarray(inputs["Wv"], dtype=f32)
    Wo = np.asarray(inputs["Wo"], dtype=f32)
    W1 = np.asarray(inputs["W1"], dtype=f32)
    W2 = np.asarray(inputs["W2"], dtype=f32)
    Wl = np.asarray(inputs["Wl"], dtype=f32)
    bq = np.asarray(inputs["bq"], dtype=f32)
    bk = np.asarray(inputs["bk"], dtype=f32)
    bv = np.asarray(inputs["bv"], dtype=f32)
    bo = np.asarray(inputs["bo"], dtype=f32)
    c1 = np.asarray(inputs["c1"], dtype=f32)
    c2 = np.asarray(inputs["c2"], dtype=f32)
    bl = np.asarray(inputs["bl"], dtype=f32)
    g1 = np.asarray(inputs["ln1_g"], dtype=f32)
    b1 = np.asarray(inputs["ln1_b"], dtype=f32)
    g2 = np.asarray(inputs["ln2_g"], dtype=f32)
    b2 = np.asarray(inputs["ln2_b"], dtype=f32)

    scale = D ** -0.5
    wq_f = np.empty((L, E, E), f32)
    wk_f = np.empty((L, E, E), f32)
    wv_f = np.empty((L, E, E), f32)
    w1_f = np.empty((L, E, DFF), f32)
    bq_f = np.empty((L, E), f32)
    bk_f = np.empty((L, E), f32)
    bv_f = np.empty((L, E), f32)
    c1_f = np.empty((L, DFF), f32)
    for l in range(L):
        wq_f[l] = g1[l][:, None] * Wq[l] * scale
        bq_f[l] = (b1[l] @ Wq[l] + bq[l]) * scale
        wk_f[l] = g1[l][:, None] * Wk[l]
        bk_f[l] = b1[l] @ Wk[l] + bk[l]
        wv_f[l] = g1[l][:, None] * Wv[l]
        bv_f[l] = b1[l] @ Wv[l] + bv[l]
        w1_f[l] = g2[l][:, None] * W1[l]
        c1_f[l] = b2[l] @ W1[l] + c1[l]

    common = {
        "embp": np.zeros((P, E), bf16),
        "pose": pos_enc,
        "maskd": np.where(np.tril(np.ones((P, P), bool)), 0.0, NEG).astype(bf16),
        "wq": wq_f.astype(bf16),
        "wk": wk_f.astype(bf16),
        "wv": wv_f.astype(bf16),
        "wl": Wl.astype(bf16),
        "bqf": np.ascontiguousarray(bq_f.reshape(L, EC, P).transpose(0, 2, 1)),
        "bkf": np.ascontiguousarray(bk_f.reshape(L, EC, P).transpose(0, 2, 1)),
        "c1f": np.ascontiguousarray(c1_f.reshape(L, FC, P).transpose(0, 2, 1)),
        "btm": np.ascontiguousarray(
            np.broadcast_to(
                np.stack([bv_f, bo, c2], axis=1)[:, :, None, :], (L, 3, P, E)
            )
        ).astype(f32),
        "blr": np.broadcast_to(bl[None, :], (P, V)).astype(f32),
    }
    common["embp"][:V, :] = emb.astype(bf16)

    if FP8_WO:
        wo8 = np.zeros((L, P, EC4, E), f32)
        wo8[:, :, 0:EC, :] = Wo.reshape(L, EC, P, E).transpose(0, 2, 1, 3)
        common["wo8"] = wo8.astype(fp8)
    else:
        common["wo"] = Wo.astype(bf16)
    if FP8_FFN:
        w18 = np.zeros((L, P, EC4, DFF), f32)
        w18[:, :, 0:EC, :] = w1_f.reshape(L, EC, P, DFF).transpose(0, 2, 1, 3)
        common["w18"] = w18.astype(fp8)
        common["w28"] = np.ascontiguousarray(
            W2.reshape(L, FC, P, E).transpose(0, 2, 1, 3)).astype(fp8)
    else:
        common["w1"] = w1_f.astype(bf16)
        common["w2"] = W2.astype(bf16)

    in_maps = []
    for c in range(N_CORES):
        tok_c = tokens[c * B_LOC : (c + 1) * B_LOC].reshape(-1)
        oht = np.zeros((P, NTOK), bf16)
        oht[tok_c, np.arange(NTOK)] = 1
        m = dict(common)
        m["oht"] = oht
        in_maps.append(m)
    return in_maps


def _biases_all_zero(inputs):
    zs = [inputs[k] for k in ("bq", "bk", "bv", "bo", "c1", "c2", "bl",
                              "ln1_b", "ln2_b")]
    return all(not np.any(np.asarray(z)) for z in zs)


def kernel(**inputs) -> np.ndarray:
    global _PROG
    zb = _biases_all_zero(inputs)
    if _PROG is None or _PROG[1] != zb:
        _PROG = (build_program(zero_bias=zb), zb)
    nc = _PROG[0]
    in_maps = _prep_host(inputs)
    res = run_bass_kernel_spmd(nc, in_maps, list(range(N_CORES)))
    outs = [res.results[c]["out"].reshape(B_LOC, T, V) for c in range(N_CORES)]
    return np.concatenate(outs, axis=0).astype(np.float32)
